# revision 21
# baseline (speedup 1.0000x reference)
"""DynamicConv Trainium2 kernel.

Math (B=1, L=2048, D=128, E=128, F=8, K1=K2=3, M=K2*D=384):
  f   = u @ proj                                   [L, F]
  kp[l,e,m] = sum_{k1,fc} f_pad[l+k1-1,fc] * W[e,k1,fc,m] + b[e,m]
  out[l,e]  = sum_{d,k2} u_pad[l+k2-1,d] * kp[l,e,d*K2+k2]

Swapping the summation order avoids materializing kp ([L,E,M] ~ 400MB):
  A_j[l,e]   = sum_{m'} patches[l,m'] * W'[m', j, e]     (j = k1*F+fc, 24 terms)
  bias_t[l,e]= sum_{m'} patches[l,m'] * b'[m', e]
  out[l,e]   = sum_j f_tap[l,j] * A_j[l,e] + bias_t[l,e]
with patches[l, (k2,d)] = u_pad[l+k2-1, d] — the patch matrix transposed is
just 3 shifted copies of u^T, so each l-tile of 128 positions needs only 3
bf16 matmuls of [128,128] x [128,424] accumulated in PSUM.  PSUM columns:
  e*25 + j   (j<24):  A_j[l,e]
  e*25 + 24        :  bias_t[l,e]
  400 + k1*8 + fc  :  f_tap[l, k1*8+fc]  (proj columns embedded in the rhs of
                      matmul k2==k1 only; the other two accumulate zeros)

Combine = broadcast multiply prod[l,e,j'] = A[l,e,j'] * f_tap[l,j'] (f
broadcast over e via a stride-0 AP; the bias slot multiplied by constant 1.0)
+ a segmented reduce over j'=25.  Work is spread over three engines:
  - GPSIMD runs the multiply for 6 of 9 groups (reading an ACT psum->sbuf
    bf16 copy), including the last two so the post-stream tail parallelizes;
  - DVE runs the remaining multiplies straight out of PSUM and ALL reduces
    (tensor_reduce is 1x-only and DVE-only: GPSIMD rejects free-axis
    reduce at the bass level and the tensor_tensor_scan opcode at the
    walrus ISA level; TensorReduce never engages the 2x packed mode even
    with all-bf16 operands).  Reduce units merge up to 2 groups (4 l-tiles)
    per instruction to amortize the ~120cy init + drain, except the last
    two units which stay small so the tail chain is short.
Input DMAs: W gates every matmul, so its 3 tap-chunks issue first (2 on
sync, 1 on scalar); u chunks follow on gpsimd/scalar/sync.  A short PE
warm-up on memset tiles fills the DMA wait and keeps the HAM activity
window hot.  Outputs are batched 8 l-tiles per DMA; the host un-permutes.

Measured context: the NEFF postamble restores all 254 declared semaphores
(~7us, invariant to program content — do not chase it), and each dma_start
costs ~600ns on its issuing engine with ~1.5us issue->transfer-start
latency; HBM is the aggregate limit for the 850KB input load.

E is sharded 8 ways (16 channels/core); u is replicated.
"""

import numpy as np
import ml_dtypes

BF16 = ml_dtypes.bfloat16

B, L, D = 1, 2048, 128
E, F = 128, 8
K1, K2 = 3, 3
M = K2 * D
NCORES = 8
EL = E // NCORES          # 16 output channels per core
NJ = K1 * F               # 24 (k1, fc) pairs
NJ1 = NJ + 1              # 25: + bias slot
NA = EL * NJ1             # 400 A/bias columns
NW = NA + NJ              # 424 total psum columns
LT = 128                  # l-tile size
NT = L // LT              # 16 l-tiles
GT = 8                    # l-tiles per output DMA group
NG = NT // GT             # output groups
UC = 4                    # l-tiles per u chunk
UCOLS = UC * LT + 2       # 514
NU = NT // UC             # 4 u chunks
PSW = 512                 # psum columns per sub-tile (bank-aligned)
NWARM = 7                 # PE warm-up matmuls (fill the input-DMA wait)
# Combine groups: (tt_on_gpsimd, [tiles]).  GPSIMD multiplies read an ACT
# psum->sbuf copy; DVE multiplies read PSUM directly (only the f columns get
# a small ACT copy).  First two groups are singles for early pipeline fill.
GROUPS = [
    (False, [0]),
    (False, [1]),
    (True, [2, 3]),
    (True, [4, 5]),
    (True, [6, 7]),
    (True, [8, 9]),
    (False, [10, 11]),
    (True, [12, 13]),
    (True, [14, 15]),
]
# Reduce units: (kind, [group indices]) over GROUPS; 'dve' = tensor_reduce,
# 'scan' = GPSIMD masked scan + ACT strided extract.  Groups inside one unit
# share a prod tile so the reduce is a single instruction.  Units must not
# cross the GT=8 output-group boundary (tiles 0-7 | 8-15).
RED_UNITS = [
    ("dve", [0, 1]),
    ("dve", [2, 3]),
    ("dve", [4]),
    ("dve", [5, 6]),
    ("dve", [7]),
    ("dve", [8]),
]
# tensor_tensor_scan is a DVE-only opcode (walrus ISA check rejects it on
# Pool), so GPSIMD cannot help with reduction at all; it gets multiplies.
USE_SCAN = False


def _build_program():
    import concourse.bass as bass
    import concourse.bacc as bacc
    import concourse.tile as tile
    from concourse import mybir

    f32 = mybir.dt.float32
    bf16 = mybir.dt.bfloat16
    nc = bacc.Bacc("TRN2", target_bir_lowering=False, debug=False)

    u_dram = nc.dram_tensor("u_padt", [D, L + 2], bf16, kind="ExternalInput")
    w_dram = nc.dram_tensor("w_aug", [D, K2 * NW], bf16, kind="ExternalInput")
    # bf16 output: DVE reduce accumulates fp32 internally and only the final
    # per-(l,e) value downcasts; makes every reduce operand 2-byte (2x_1p
    # eligible) and halves the output DMA.  Host upcasts to f32.
    o_dram = nc.dram_tensor("out", [NG, D, GT * EL], bf16, kind="ExternalOutput")

    with tile.TileContext(nc) as tc:
        import contextlib

        with contextlib.ExitStack() as ctx:
            const_pool = ctx.enter_context(tc.tile_pool(name="const", bufs=1))
            psum_pool = ctx.enter_context(
                tc.tile_pool(name="psum", bufs=4, space="PSUM")
            )
            fpool = ctx.enter_context(tc.tile_pool(name="ftile", bufs=4))
            prodp = ctx.enter_context(tc.tile_pool(name="prod", bufs=3))
            outp = ctx.enter_context(tc.tile_pool(name="outt", bufs=2))

            u_sbs = []
            for g in range(NU):
                u_g = const_pool.tile([D, UCOLS], bf16, tag=f"u{g}", name=f"u{g}")
                u_sbs.append(u_g)
            w_sb = const_pool.tile([D, K2 * NW], bf16)

            # W before u on every queue (it gates all matmul groups); u0
            # early so tile 0 starts as soon as possible.
            def dma_w(k, eng):
                eng.dma_start(
                    out=w_sb[:, k * NW : (k + 1) * NW],
                    in_=w_dram[:, k * NW : (k + 1) * NW],
                )

            def dma_u(g, eng):
                eng.dma_start(
                    out=u_sbs[g][:],
                    in_=u_dram[:, g * UC * LT : g * UC * LT + UCOLS],
                )

            with tc.high_priority():
                dma_w(0, nc.sync)
                dma_w(2, nc.scalar)
                dma_u(0, nc.gpsimd)
                dma_w(1, nc.sync)
                dma_u(1, nc.scalar)
                dma_u(2, nc.gpsimd)
                dma_u(3, nc.sync)

            # PE warm-up on memset tiles while the input DMAs stream.
            warm_in = const_pool.tile([D, LT], bf16, name="warm_in")
            warm_rhs = const_pool.tile([D, PSW], bf16, name="warm_rhs")
            nc.vector.memset(warm_in[:], 0.0)
            nc.vector.memset(warm_rhs[:], 0.0)
            warm_ps = psum_pool.tile([LT, 2, PSW], f32, tag="ps", name="warm_ps")
            for i in range(NWARM):
                nc.tensor.matmul(
                    warm_ps[:, 0, :],
                    warm_in[:],
                    warm_rhs[:],
                    start=(i == 0),
                    stop=(i == NWARM - 1),
                )

            # Scan mask: 1.0 everywhere, 0.0 at each 25-segment start, so
            # state = mask*state + prod restarts the running sum per (g,e).
            mask = None
            if USE_SCAN:
                mask = const_pool.tile([LT, 4 * EL, NJ1], bf16, name="scanmask")
                nc.gpsimd.memset(mask[:], 1.0)
                nc.gpsimd.memset(mask[:, :, 0:1], 0.0)

            def with_e_bcast(ap):
                # [128, QT, 25] -> [128, QT, EL, 25]: stride-0 bcast over e
                return bass.AP(
                    tensor=ap.tensor,
                    offset=ap.offset,
                    ap=[ap.ap[0], ap.ap[1], [0, EL], ap.ap[2]],
                )

            # prod tiles are shared per reduce unit
            unit_of_group = {}
            unit_q0 = {}
            for ui, (kind, gis) in enumerate(RED_UNITS):
                off = 0
                for gi in gis:
                    unit_of_group[gi] = ui
                    unit_q0[gi] = off
                    off += len(GROUPS[gi][1])

            o_big = None
            prod_tiles = {}
            for gi, (tt_gp, tiles) in enumerate(GROUPS):
                q = len(tiles)
                if tiles[0] % GT == 0:
                    o_big = outp.tile([LT, GT, EL], bf16, name="o_big")
                gout = tiles[0] // GT
                ps = psum_pool.tile([LT, q, PSW], f32, tag="ps", name="ps")
                for g, t in enumerate(tiles):
                    u_g = u_sbs[t // UC]
                    lo = (t % UC) * LT
                    for k in range(K2):
                        nc.tensor.matmul(
                            ps[:, g, 0:NW],
                            u_g[:, lo + k : lo + k + LT],
                            w_sb[:, k * NW : (k + 1) * NW],
                            start=(k == 0),
                            stop=(k == K2 - 1),
                        )

                ui = unit_of_group[gi]
                kind, gis = RED_UNITS[ui]
                uq = sum(len(GROUPS[g][1]) for g in gis)
                if ui not in prod_tiles:
                    prod_tiles[ui] = prodp.tile(
                        [LT, uq, EL, NJ1], bf16, name=f"prod{ui}"
                    )
                prod = prod_tiles[ui]
                q0 = unit_q0[gi]
                pslice = prod[:, q0 : q0 + q]

                if tt_gp:
                    # ACT copies the psum group (A + f cols) to SBUF bf16;
                    # GPSIMD does the broadcast multiply from SBUF.
                    asb = fpool.tile([LT, q, NW + 1], bf16, tag="asb", name="asb")
                    nc.vector.memset(asb[:, :, NW : NW + 1], 1.0)
                    nc.scalar.copy(out=asb[:, :, 0:NW], in_=ps[:, :, 0:NW])
                    nc.gpsimd.tensor_tensor(
                        out=pslice,
                        in0=asb[:, :, 0:NA].rearrange(
                            "q g (e j) -> q g e j", j=NJ1
                        ),
                        in1=with_e_bcast(asb[:, :, NA : NW + 1]),
                        op=mybir.AluOpType.mult,
                    )
                else:
                    # DVE multiplies straight from PSUM.
                    fsb = fpool.tile([LT, q, NJ1], f32, tag="fsb", name="fsb")
                    nc.vector.memset(fsb[:, :, NJ:NJ1], 1.0)
                    nc.scalar.copy(out=fsb[:, :, 0:NJ], in_=ps[:, :, NA:NW])
                    nc.vector.tensor_tensor(
                        out=pslice,
                        in0=ps[:, :, 0:NA].rearrange("q g (e j) -> q g e j", j=NJ1),
                        in1=with_e_bcast(fsb[:]),
                        op=mybir.AluOpType.mult,
                    )

                if gi == gis[-1]:
                    # last group of the unit: run the reduce
                    first_tile = GROUPS[gis[0]][1][0]
                    s0 = first_tile % GT
                    if kind == "dve" or not USE_SCAN:
                        with nc.allow_low_precision(
                            "reduce accumulates fp32 internally; only the "
                            "final per-(l,e) value is bf16"
                        ):
                            nc.vector.reduce_sum(
                                out=o_big[:, s0 : s0 + uq, :],
                                in_=prod[:],
                                axis=mybir.AxisListType.X,
                            )
                    else:
                        scanout = prodp.tile(
                            [LT, uq * EL, NJ1], bf16, tag="scano", name="scano"
                        )
                        nc.gpsimd.tensor_tensor_scan(
                            out=scanout[:].rearrange("p a b -> p (a b)"),
                            data0=mask[:, 0 : uq * EL, :].rearrange(
                                "p a b -> p (a b)"
                            ),
                            data1=prod[:].rearrange("p a b c -> p (a b c)"),
                            initial=0.0,
                            op0=mybir.AluOpType.mult,
                            op1=mybir.AluOpType.add,
                        )
                        nc.scalar.copy(
                            out=o_big[:, s0 : s0 + uq, :].rearrange(
                                "p a b -> p (a b)"
                            ),
                            in_=scanout[:, :, NJ : NJ + 1].rearrange(
                                "p a b -> p (a b)"
                            ),
                        )
                if tiles[-1] % GT == GT - 1:
                    nc.sync.dma_start(out=o_dram[gout], in_=o_big[:])

    nc.compile()
    return nc


def _prep_inputs(u, proj, conv_w, conv_b):
    """Host-side layout prep: reshuffle + bf16 rounding only."""
    u_padt = np.zeros((D, L + 2), BF16)
    u_padt[:, 1 : L + 1] = np.ascontiguousarray(u[0].T).astype(BF16)

    in_maps = []
    for c in range(NCORES):
        e0 = c * EL
        w_aug = np.zeros((K2, D, NW), np.float32)
        # conv weights: m = d*K2 + k2 (in_channel-major, tap-minor)
        cw = conv_w[e0 : e0 + EL].reshape(EL, K1, F, D, K2)
        wmain = cw.transpose(4, 3, 0, 1, 2).reshape(K2, D, EL, NJ)
        wa = w_aug[:, :, :NA].reshape(K2, D, EL, NJ1)
        wa[:, :, :, :NJ] = wmain
        # bias at j' = 24 (multiplied by the constant-1 f slot)
        cb = conv_b[e0 : e0 + EL, 0, :, 0].reshape(EL, D, K2)
        wa[:, :, :, NJ] = cb.transpose(2, 1, 0)
        # proj columns: only in the k2 == k1 matmul
        for k in range(K2):
            w_aug[k, :, NA + k * F : NA + (k + 1) * F] = proj
        w_flat = w_aug.transpose(1, 0, 2).reshape(D, K2 * NW).astype(BF16)
        in_maps.append(
            {"u_padt": u_padt, "w_aug": np.ascontiguousarray(w_flat)}
        )
    return in_maps


_PROGRAM_CACHE = {}


def kernel(
    u,
    kernel_params_feat_proj,
    kernel_params_conv_weights,
    kernel_params_conv_bias,
):
    from concourse.bass_utils import run_bass_kernel_spmd

    u = np.asarray(u, np.float32)
    proj = np.asarray(kernel_params_feat_proj, np.float32)
    conv_w = np.asarray(kernel_params_conv_weights, np.float32)
    conv_b = np.asarray(kernel_params_conv_bias, np.float32)

    if "nc" not in _PROGRAM_CACHE:
        _PROGRAM_CACHE["nc"] = _build_program()
    nc = _PROGRAM_CACHE["nc"]

    in_maps = _prep_inputs(u, proj, conv_w, conv_b)
    res = run_bass_kernel_spmd(nc, in_maps, list(range(NCORES)))

    out = np.empty((B, L, E), np.float32)
    for c in range(NCORES):
        # o_dram [NG, 128, GT, EL] with l = (g*GT + t)*128 + l_sub
        arr = res.results[c]["out"].astype(np.float32).reshape(NG, LT, GT, EL)
        arr = arr.transpose(0, 2, 1, 3).reshape(L, EL)
        out[0, :, c * EL : (c + 1) * EL] = arr
    return out


# revision 22
# speedup vs baseline: 1.0259x; 1.0259x over previous
"""DynamicConv Trainium2 kernel.

Math (B=1, L=2048, D=128, E=128, F=8, K1=K2=3, M=K2*D=384):
  f   = u @ proj                                   [L, F]
  kp[l,e,m] = sum_{k1,fc} f_pad[l+k1-1,fc] * W[e,k1,fc,m] + b[e,m]
  out[l,e]  = sum_{d,k2} u_pad[l+k2-1,d] * kp[l,e,d*K2+k2]

Swapping the summation order avoids materializing kp ([L,E,M] ~ 400MB):
  A_j[l,e]   = sum_{m'} patches[l,m'] * W'[m', j, e]     (j = k1*F+fc, 24 terms)
  bias_t[l,e]= sum_{m'} patches[l,m'] * b'[m', e]
  out[l,e]   = sum_j f_tap[l,j] * A_j[l,e] + bias_t[l,e]
with patches[l, (k2,d)] = u_pad[l+k2-1, d] — the patch matrix transposed is
just 3 shifted copies of u^T, so each l-tile of 128 positions needs only 3
bf16 matmuls of [128,128] x [128,424] accumulated in PSUM.  PSUM columns:
  e*25 + j   (j<24):  A_j[l,e]
  e*25 + 24        :  bias_t[l,e]
  400 + k1*8 + fc  :  f_tap[l, k1*8+fc]  (proj columns embedded in the rhs of
                      matmul k2==k1 only; the other two accumulate zeros)

Combine = broadcast multiply prod[l,e,j'] = A[l,e,j'] * f_tap[l,j'] (f
broadcast over e via a stride-0 AP; the bias slot multiplied by constant 1.0)
+ a segmented reduce over j'=25.  Work is spread over three engines:
  - GPSIMD runs the multiply for 6 of 9 groups (reading an ACT psum->sbuf
    bf16 copy), including the last two so the post-stream tail parallelizes;
  - DVE runs the remaining multiplies straight out of PSUM and ALL reduces
    (tensor_reduce is 1x-only and DVE-only: GPSIMD rejects free-axis
    reduce at the bass level and the tensor_tensor_scan opcode at the
    walrus ISA level; TensorReduce never engages the 2x packed mode even
    with all-bf16 operands).  Reduce units merge up to 2 groups (4 l-tiles)
    per instruction to amortize the ~120cy init + drain, except the last
    two units which stay small so the tail chain is short.
Input DMAs: W gates every matmul, so its 3 tap-chunks issue first (2 on
sync, 1 on scalar); u chunks follow on gpsimd/scalar/sync.  A short PE
warm-up on memset tiles fills the DMA wait and keeps the HAM activity
window hot.  Outputs are batched 8 l-tiles per DMA; the host un-permutes.

Measured context: the NEFF postamble restores all 254 declared semaphores
(~7us, invariant to program content — do not chase it), and each dma_start
costs ~600ns on its issuing engine with ~1.5us issue->transfer-start
latency; HBM is the aggregate limit for the 850KB input load.

E is sharded 8 ways (16 channels/core); u is replicated.
"""

import numpy as np
import ml_dtypes

BF16 = ml_dtypes.bfloat16

B, L, D = 1, 2048, 128
E, F = 128, 8
K1, K2 = 3, 3
M = K2 * D
NCORES = 8
EL = E // NCORES          # 16 output channels per core
NJ = K1 * F               # 24 (k1, fc) pairs
NJ1 = NJ + 1              # 25: + bias slot
NA = EL * NJ1             # 400 A/bias columns
NW = NA + NJ              # 424 total psum columns
LT = 128                  # l-tile size
NT = L // LT              # 16 l-tiles
GT = 8                    # l-tiles per output DMA group
NG = NT // GT             # output groups
UC = 4                    # l-tiles per u chunk
UCOLS = UC * LT + 2       # 514
NU = NT // UC             # 4 u chunks
PSW = 512                 # psum columns per sub-tile (bank-aligned)
NWARM = 7                 # PE warm-up matmuls (fill the input-DMA wait)
# Combine groups: (tt_on_gpsimd, [tiles]).  GPSIMD multiplies read an ACT
# psum->sbuf copy; DVE multiplies read PSUM directly (only the f columns get
# a small ACT copy).  First two groups are singles for early pipeline fill.
GROUPS = [
    (False, [0]),
    (False, [1]),
    (False, [2, 3]),
    (True, [4, 5]),
    (True, [6, 7]),
    (True, [8, 9]),
    (False, [10, 11]),
    (True, [12, 13]),
    (True, [14, 15]),
]
# Reduce units: (kind, [group indices]) over GROUPS; 'dve' = tensor_reduce,
# 'scan' = GPSIMD masked scan + ACT strided extract.  Groups inside one unit
# share a prod tile so the reduce is a single instruction.  Units must not
# cross the GT=8 output-group boundary (tiles 0-7 | 8-15).
RED_UNITS = [
    ("dve", [0, 1]),
    ("dve", [2, 3]),
    ("dve", [4]),
    ("dve", [5, 6]),
    ("dve", [7]),
    ("dve", [8]),
]
# tensor_tensor_scan is a DVE-only opcode (walrus ISA check rejects it on
# Pool), so GPSIMD cannot help with reduction at all; it gets multiplies.
USE_SCAN = False


def _build_program():
    import concourse.bass as bass
    import concourse.bacc as bacc
    import concourse.tile as tile
    from concourse import mybir

    f32 = mybir.dt.float32
    bf16 = mybir.dt.bfloat16
    nc = bacc.Bacc("TRN2", target_bir_lowering=False, debug=False)

    u_dram = nc.dram_tensor("u_padt", [D, L + 2], bf16, kind="ExternalInput")
    w_dram = nc.dram_tensor("w_aug", [D, K2 * NW], bf16, kind="ExternalInput")
    # bf16 output: DVE reduce accumulates fp32 internally and only the final
    # per-(l,e) value downcasts; makes every reduce operand 2-byte (2x_1p
    # eligible) and halves the output DMA.  Host upcasts to f32.
    o_dram = nc.dram_tensor("out", [NG, D, GT * EL], bf16, kind="ExternalOutput")

    with tile.TileContext(nc) as tc:
        import contextlib

        with contextlib.ExitStack() as ctx:
            const_pool = ctx.enter_context(tc.tile_pool(name="const", bufs=1))
            psum_pool = ctx.enter_context(
                tc.tile_pool(name="psum", bufs=4, space="PSUM")
            )
            fpool = ctx.enter_context(tc.tile_pool(name="ftile", bufs=4))
            prodp = ctx.enter_context(tc.tile_pool(name="prod", bufs=3))
            outp = ctx.enter_context(tc.tile_pool(name="outt", bufs=2))

            u_sbs = []
            for g in range(NU):
                u_g = const_pool.tile([D, UCOLS], bf16, tag=f"u{g}", name=f"u{g}")
                u_sbs.append(u_g)
            w_sb = const_pool.tile([D, K2 * NW], bf16)

            # W before u on every queue (it gates all matmul groups); u0
            # early so tile 0 starts as soon as possible.
            def dma_w(k, eng):
                eng.dma_start(
                    out=w_sb[:, k * NW : (k + 1) * NW],
                    in_=w_dram[:, k * NW : (k + 1) * NW],
                )

            def dma_u(g, eng):
                eng.dma_start(
                    out=u_sbs[g][:],
                    in_=u_dram[:, g * UC * LT : g * UC * LT + UCOLS],
                )

            with tc.high_priority():
                dma_w(0, nc.sync)
                dma_w(2, nc.scalar)
                dma_u(0, nc.gpsimd)
                dma_w(1, nc.sync)
                dma_u(1, nc.scalar)
                dma_u(2, nc.gpsimd)
                dma_u(3, nc.sync)

            # PE warm-up on memset tiles while the input DMAs stream.
            warm_in = const_pool.tile([D, LT], bf16, name="warm_in")
            warm_rhs = const_pool.tile([D, PSW], bf16, name="warm_rhs")
            nc.vector.memset(warm_in[:], 0.0)
            nc.vector.memset(warm_rhs[:], 0.0)
            warm_ps = psum_pool.tile([LT, 2, PSW], f32, tag="ps", name="warm_ps")
            for i in range(NWARM):
                nc.tensor.matmul(
                    warm_ps[:, 0, :],
                    warm_in[:],
                    warm_rhs[:],
                    start=(i == 0),
                    stop=(i == NWARM - 1),
                )

            # Scan mask: 1.0 everywhere, 0.0 at each 25-segment start, so
            # state = mask*state + prod restarts the running sum per (g,e).
            mask = None
            if USE_SCAN:
                mask = const_pool.tile([LT, 4 * EL, NJ1], bf16, name="scanmask")
                nc.gpsimd.memset(mask[:], 1.0)
                nc.gpsimd.memset(mask[:, :, 0:1], 0.0)

            def with_e_bcast(ap):
                # [128, QT, 25] -> [128, QT, EL, 25]: stride-0 bcast over e
                return bass.AP(
                    tensor=ap.tensor,
                    offset=ap.offset,
                    ap=[ap.ap[0], ap.ap[1], [0, EL], ap.ap[2]],
                )

            # prod tiles are shared per reduce unit
            unit_of_group = {}
            unit_q0 = {}
            for ui, (kind, gis) in enumerate(RED_UNITS):
                off = 0
                for gi in gis:
                    unit_of_group[gi] = ui
                    unit_q0[gi] = off
                    off += len(GROUPS[gi][1])

            o_big = None
            prod_tiles = {}
            for gi, (tt_gp, tiles) in enumerate(GROUPS):
                q = len(tiles)
                if tiles[0] % GT == 0:
                    o_big = outp.tile([LT, GT, EL], bf16, name="o_big")
                gout = tiles[0] // GT
                ps = psum_pool.tile([LT, q, PSW], f32, tag="ps", name="ps")
                for g, t in enumerate(tiles):
                    u_g = u_sbs[t // UC]
                    lo = (t % UC) * LT
                    for k in range(K2):
                        nc.tensor.matmul(
                            ps[:, g, 0:NW],
                            u_g[:, lo + k : lo + k + LT],
                            w_sb[:, k * NW : (k + 1) * NW],
                            start=(k == 0),
                            stop=(k == K2 - 1),
                        )

                ui = unit_of_group[gi]
                kind, gis = RED_UNITS[ui]
                uq = sum(len(GROUPS[g][1]) for g in gis)
                if ui not in prod_tiles:
                    prod_tiles[ui] = prodp.tile(
                        [LT, uq, EL, NJ1], bf16, name=f"prod{ui}"
                    )
                prod = prod_tiles[ui]
                q0 = unit_q0[gi]
                pslice = prod[:, q0 : q0 + q]

                if tt_gp:
                    # ACT copies the psum group (A + f cols) to SBUF bf16;
                    # GPSIMD does the broadcast multiply from SBUF.
                    asb = fpool.tile([LT, q, NW + 1], bf16, tag="asb", name="asb")
                    nc.vector.memset(asb[:, :, NW : NW + 1], 1.0)
                    nc.scalar.copy(out=asb[:, :, 0:NW], in_=ps[:, :, 0:NW])
                    nc.gpsimd.tensor_tensor(
                        out=pslice,
                        in0=asb[:, :, 0:NA].rearrange(
                            "q g (e j) -> q g e j", j=NJ1
                        ),
                        in1=with_e_bcast(asb[:, :, NA : NW + 1]),
                        op=mybir.AluOpType.mult,
                    )
                else:
                    # DVE multiplies straight from PSUM.
                    fsb = fpool.tile([LT, q, NJ1], f32, tag="fsb", name="fsb")
                    nc.vector.memset(fsb[:, :, NJ:NJ1], 1.0)
                    nc.scalar.copy(out=fsb[:, :, 0:NJ], in_=ps[:, :, NA:NW])
                    nc.vector.tensor_tensor(
                        out=pslice,
                        in0=ps[:, :, 0:NA].rearrange("q g (e j) -> q g e j", j=NJ1),
                        in1=with_e_bcast(fsb[:]),
                        op=mybir.AluOpType.mult,
                    )

                if gi == gis[-1]:
                    # last group of the unit: run the reduce
                    first_tile = GROUPS[gis[0]][1][0]
                    s0 = first_tile % GT
                    if kind == "dve" or not USE_SCAN:
                        with nc.allow_low_precision(
                            "reduce accumulates fp32 internally; only the "
                            "final per-(l,e) value is bf16"
                        ):
                            nc.vector.reduce_sum(
                                out=o_big[:, s0 : s0 + uq, :],
                                in_=prod[:],
                                axis=mybir.AxisListType.X,
                            )
                    else:
                        scanout = prodp.tile(
                            [LT, uq * EL, NJ1], bf16, tag="scano", name="scano"
                        )
                        nc.gpsimd.tensor_tensor_scan(
                            out=scanout[:].rearrange("p a b -> p (a b)"),
                            data0=mask[:, 0 : uq * EL, :].rearrange(
                                "p a b -> p (a b)"
                            ),
                            data1=prod[:].rearrange("p a b c -> p (a b c)"),
                            initial=0.0,
                            op0=mybir.AluOpType.mult,
                            op1=mybir.AluOpType.add,
                        )
                        nc.scalar.copy(
                            out=o_big[:, s0 : s0 + uq, :].rearrange(
                                "p a b -> p (a b)"
                            ),
                            in_=scanout[:, :, NJ : NJ + 1].rearrange(
                                "p a b -> p (a b)"
                            ),
                        )
                if tiles[-1] % GT == GT - 1:
                    nc.sync.dma_start(out=o_dram[gout], in_=o_big[:])

    nc.compile()
    return nc


def _prep_inputs(u, proj, conv_w, conv_b):
    """Host-side layout prep: reshuffle + bf16 rounding only."""
    u_padt = np.zeros((D, L + 2), BF16)
    u_padt[:, 1 : L + 1] = np.ascontiguousarray(u[0].T).astype(BF16)

    in_maps = []
    for c in range(NCORES):
        e0 = c * EL
        w_aug = np.zeros((K2, D, NW), np.float32)
        # conv weights: m = d*K2 + k2 (in_channel-major, tap-minor)
        cw = conv_w[e0 : e0 + EL].reshape(EL, K1, F, D, K2)
        wmain = cw.transpose(4, 3, 0, 1, 2).reshape(K2, D, EL, NJ)
        wa = w_aug[:, :, :NA].reshape(K2, D, EL, NJ1)
        wa[:, :, :, :NJ] = wmain
        # bias at j' = 24 (multiplied by the constant-1 f slot)
        cb = conv_b[e0 : e0 + EL, 0, :, 0].reshape(EL, D, K2)
        wa[:, :, :, NJ] = cb.transpose(2, 1, 0)
        # proj columns: only in the k2 == k1 matmul
        for k in range(K2):
            w_aug[k, :, NA + k * F : NA + (k + 1) * F] = proj
        w_flat = w_aug.transpose(1, 0, 2).reshape(D, K2 * NW).astype(BF16)
        in_maps.append(
            {"u_padt": u_padt, "w_aug": np.ascontiguousarray(w_flat)}
        )
    return in_maps


_PROGRAM_CACHE = {}


def kernel(
    u,
    kernel_params_feat_proj,
    kernel_params_conv_weights,
    kernel_params_conv_bias,
):
    from concourse.bass_utils import run_bass_kernel_spmd

    u = np.asarray(u, np.float32)
    proj = np.asarray(kernel_params_feat_proj, np.float32)
    conv_w = np.asarray(kernel_params_conv_weights, np.float32)
    conv_b = np.asarray(kernel_params_conv_bias, np.float32)

    if "nc" not in _PROGRAM_CACHE:
        _PROGRAM_CACHE["nc"] = _build_program()
    nc = _PROGRAM_CACHE["nc"]

    in_maps = _prep_inputs(u, proj, conv_w, conv_b)
    res = run_bass_kernel_spmd(nc, in_maps, list(range(NCORES)))

    out = np.empty((B, L, E), np.float32)
    for c in range(NCORES):
        # o_dram [NG, 128, GT, EL] with l = (g*GT + t)*128 + l_sub
        arr = res.results[c]["out"].astype(np.float32).reshape(NG, LT, GT, EL)
        arr = arr.transpose(0, 2, 1, 3).reshape(L, EL)
        out[0, :, c * EL : (c + 1) * EL] = arr
    return out
